# revision 43
# baseline (speedup 1.0000x reference)
"""Trainium2 Bass kernel for nn_Decoder (GRU decoder with dual attention).

Strategy (8 NeuronCores, batch-parallel, zero collectives). v2 rewrite of
the baseline, targeting the measured bottleneck: per-matmul LDWEIGHTS /
dispatch cost (~80ns) x 242 matmuls/step, plus a ~4us per-step PE idle
gap that kept HAM at half clock.

Key changes vs baseline:
  - M=8 stationaries everywhere (LDWEIGHTS cost ~ columns): th slices
    [128,8] feed col-group-tiled matmuls at tile_position (0,32j).
  - rr-pairs merged into single matmuls via 2-block (3D) APs: scores
    N=384, pose-woh N=272 (woh stored duplicated), output DMAs 2/step.
  - All gate biases folded into the W_ih ones-row (row 135); optional
    b_hh n-part mini-round only compiled when b_hh != 0.
  - Softmax without max-subtraction (|scores| <= ~21 for this problem,
    exp fits fp32 with huge margin), per-segment independent tiles so
    the 4 exps pipeline instead of serializing, normalization fused into
    the bf16 cast (tensor_scalar_mul with per-partition 1/sum).
  - tanh(x) == 2*sigmoid(2x)-1 so ACT uses only Sigmoid+Exp tables.
  - Cross-step overlap: next step's W_hh rounds are emitted right after
    this step's score matmuls so the PE stays busy during softmax; PSUM
    rz/nn pools are double-buffered. Eliminates the HAM re-throttle gap.
  - p2e column 135 := 1.0 (b_out ones-col) so the transposed pose
    carries the gi-bias ones row for free (softmax weights sum to 1).

Layouts (per core, BL=8 batches):
  h4  [128, 256] fp32: row 32*j+b = h[b, j*256:(j+1)*256]  (b<8 valid)
  th[half] [128,128] bf16: th[half][r, 32*q+b] = h[b, q*256+half*128+r]
    => contraction tile k uses lhsT th[k%2][:, 32*(k//2) : 32*(k//2)+8]
  gate cols grouped: group j = [r_j | z_j | n_j] each 256 wide;
    rz psum [128,512] = r|z for group j at rows 32j; nn psum [128,512]:
    cols 0:256 = gh_n, 256:512 = gi_n.
  projT[k] [128, 8*192] bf16: projT[k][r, b*192+c]: c<128 enc proj at
    s=c; c in 128:192 word proj at wl=c-128; h-dim k*128+r; bias folded.
  p2e [128, 8*136]: p2e[s, b*136+o] = (W_oc @ enc_proj + b_out)[o],
    col o=135 is 1.0 (ones trick).  p2w2 [128, 8*136]: word analogue
    duplicated on partitions 0:64 and 64:128.
"""

import os
import sys

sys.path.insert(0, "/opt/trn_rl_repo")

import numpy as np

S, B, E, H, O, WL, PL = 128, 64, 1024, 1024, 135, 64, 32
NCORES = 8
BL = B // NCORES          # 8 batches per core
G = 4                     # gate column groups
GH = H // G               # 256 hidden dims per group
OP = 136                  # padded pose dim (135 + ones col)
SCE, SCW = S, WL
SC = SCE + SCW            # 192 score cols per batch

_progs = {}


def _group_cols():
    """Column permutation of the 3H gate dim into G groups of [r|z|n]."""
    cols = []
    for j in range(G):
        h0 = j * GH
        cols.extend(range(h0, h0 + GH))
        cols.extend(range(H + h0, H + h0 + GH))
        cols.extend(range(2 * H + h0, 2 * H + h0 + GH))
    return np.asarray(cols)


def _body(tc, outs, ins, T, PLc, has_bhh):
    from concourse import mybir
    from concourse.masks import make_identity

    nc = tc.nc
    f32 = mybir.dt.float32
    bf16 = mybir.dt.bfloat16
    EXP = mybir.ActivationFunctionType.Exp
    SIG = mybir.ActivationFunctionType.Sigmoid
    MULT = mybir.AluOpType.mult
    ADD = mybir.AluOpType.add

    def mm(out, lhsT, rhs, start, stop, tp=None):
        nc.tensor.matmul(out, lhsT, rhs, start=start, stop=stop,
                         tile_position=tp, skip_group_check=True)

    import contextlib
    ctx = contextlib.ExitStack()
    with ctx:
        wp = ctx.enter_context(tc.tile_pool(name="wp", bufs=1))
        work = ctx.enter_context(tc.tile_pool(name="work", bufs=2))
        gt = ctx.enter_context(tc.tile_pool(name="gt", bufs=2))
        ps_g = ctx.enter_context(tc.tile_pool(name="ps_g", bufs=2, space="PSUM"))
        ps_n = ctx.enter_context(tc.tile_pool(name="ps_n", bufs=2, space="PSUM"))
        ps_s = ctx.enter_context(tc.tile_pool(name="ps_s", bufs=1, space="PSUM"))
        ps_p = ctx.enter_context(tc.tile_pool(name="ps_p", bufs=1, space="PSUM"))
        ps_t = ctx.enter_context(tc.tile_pool(name="ps_t", bufs=2, space="PSUM"))

        # ---------------- persistent weights ----------------
        whh = []
        for k in range(8):
            t = wp.tile([128, 3 * H], bf16, tag=f"whh{k}")
            nc.sync.dma_start(out=t, in_=ins["whh_t"][k * 128:(k + 1) * 128, :])
            whh.append(t)
        wih0 = wp.tile([128, 3 * H], bf16, tag="wih0")
        nc.sync.dma_start(out=wih0, in_=ins["wih_t"][0:128, :])
        wih1 = wp.tile([8, 3 * H], bf16, tag="wih1")
        nc.sync.dma_start(out=wih1, in_=ins["wih_t"][128:136, :])
        woh2 = []
        for k in range(8):
            t = wp.tile([128, 2 * OP], bf16, tag=f"woh2_{k}")
            nc.sync.dma_start(out=t, in_=ins["woh2_t"][k * 128:(k + 1) * 128, :])
            woh2.append(t)
        posesT0 = wp.tile([128, PLc * BL], bf16, tag="posesT0")
        nc.sync.dma_start(out=posesT0, in_=ins["poses_t"][0:128, :])
        posesT1 = wp.tile([8, PLc * BL], bf16, tag="posesT1")
        nc.sync.dma_start(out=posesT1, in_=ins["poses_t"][128:136, :])
        if has_bhh:
            bhh_n = wp.tile([1, H], bf16, tag="bhh_n")
            nc.sync.dma_start(out=bhh_n, in_=ins["bhh_n"][:, :])
        bout_sb = wp.tile([1, OP], bf16, tag="bout_sb")
        nc.sync.dma_start(out=bout_sb, in_=ins["bout"][:, :])

        ident = wp.tile([128, 128], f32, tag="ident")
        make_identity(nc, ident[:, :])
        identb = wp.tile([128, 128], bf16, tag="identb")
        nc.vector.tensor_copy(identb, ident)
        ones1 = wp.tile([1, 128], bf16, tag="ones1")
        nc.vector.memset(ones1, 1.0)

        projT = [wp.tile([128, BL * SC], bf16, tag=f"projT{m}", name=f"projT{m}")
                 for m in range(8)]
        p2e = wp.tile([128, BL * OP], bf16, tag="p2e")
        p2w2 = wp.tile([128, BL * OP], bf16, tag="p2w2")

        # ---------------- prologue input DMAs (start early) --------------
        # enc activations: [1025, 1024] in 2 col-chunks of 512, 9 row tiles
        xe = [[None] * 9 for _ in range(2)]
        for c in range(2):
            for k in range(9):
                kp = 128 if k < 8 else 1
                t = work.tile([128, 512], bf16, tag="xe", bufs=18,
                              name=f"xe{c}_{k}")
                nc.sync.dma_start(
                    out=t[:kp, :],
                    in_=ins["xt_enc"][k * 128:k * 128 + kp,
                                      c * 512:(c + 1) * 512])
                xe[c][k] = t
        xw0 = work.tile([128, 512], bf16, tag="xw0", bufs=1)
        nc.sync.dma_start(out=xw0, in_=ins["xt_word"][0:128, :])
        xw1 = work.tile([73, 512], bf16, tag="xw1", bufs=1)
        nc.sync.dma_start(out=xw1, in_=ins["xt_word"][128:201, :])
        # watt resident during prologue: 9 x [128, 1024]
        watt = []
        for k in range(9):
            kp = 128 if k < 8 else 1
            t = work.tile([128, H], bf16, tag="watt", bufs=9, name=f"watt{k}")
            nc.sync.dma_start(out=t[:kp, :],
                              in_=ins["watt_t"][k * 128:k * 128 + kp, :])
            watt.append(t)
        wwatt = []
        for k in range(2):
            kp = 128 if k == 0 else 73
            t = work.tile([128, H], bf16, tag="wwatt", bufs=2, name=f"wwatt{k}")
            nc.sync.dma_start(out=t[:kp, :],
                              in_=ins["wwatt_t"][k * 128:k * 128 + kp, :])
            wwatt.append(t)
        wocw = [work.tile([128, OP], bf16, tag="wocw", bufs=16, name=f"wocw{k}")
                for k in range(8)]
        for k in range(8):
            nc.sync.dma_start(out=wocw[k], in_=ins["woc_t"][k * 128:(k + 1) * 128, :])
        woww = [work.tile([128, OP], bf16, tag="wocw", bufs=16, name=f"woww{k}")
                for k in range(8)]
        for k in range(8):
            nc.sync.dma_start(out=woww[k], in_=ins["wow_t"][k * 128:(k + 1) * 128, :])
        ehk = []
        for k in range(9):
            kp = 128 if k < 8 else 1
            t = work.tile([128, BL], bf16, tag="ehk", bufs=9, name=f"ehk{k}")
            nc.sync.dma_start(out=t[:kp, :],
                              in_=ins["eht"][k * 128:k * 128 + kp, :])
            ehk.append(t)
        wed = []
        for k in range(9):
            kp = 128 if k < 8 else 1
            t = work.tile([128, H], bf16, tag="wed", bufs=9, name=f"wed{k}")
            nc.sync.dma_start(out=t[:kp, :],
                              in_=ins["wed_t"][k * 128:k * 128 + kp, :])
            wed.append(t)

        # ---------------- h0 ----------------
        h0p = ps_s.tile([128, 384], f32, tag="sc", name="h0p")
        for k in range(9):
            kp = 128 if k < 8 else 1
            for j in range(G):
                mm(h0p[32 * j:32 * j + 8, 0:256], ehk[k][:kp, :],
                   wed[k][:kp, j * GH:(j + 1) * GH],
                   start=(k == 0), stop=(k == 8), tp=(0, 32 * j))
        h4 = (gt.tile([128, 128], f32, tag="h4a", name="h4a_init"),
              gt.tile([128, 128], f32, tag="h4b", name="h4b_init"))
        nc.vector.tensor_copy(h4[0], h0p[:, 0:128])
        nc.vector.tensor_copy(h4[1], h0p[:, 128:256])

        # ---------------- recurrent helpers ----------------
        f32r = mybir.dt.float32r
        R = bool(int(os.environ.get("R32", "0")))

        def th_lhsT(th, k):
            q = k // 2
            sl = th[k % 2][:, 32 * q:32 * q + 8]
            return sl.bitcast(f32r) if R else sl

        def transpose_h(h4t):
            th = []
            for half in range(2):
                tp = ps_t.tile([128, 128], f32, tag="tp")
                nc.tensor.transpose(tp, h4t[half], ident)
                sb = gt.tile([128, 128], f32 if R else bf16, tag=f"th{half}")
                nc.vector.tensor_copy(sb, tp)
                th.append(sb)
            return th

        QD = bool(int(os.environ.get("QD", "1")))
        G2 = bool(int(os.environ.get("G2", "1")))
        SM2 = bool(int(os.environ.get("SM2", "1")))
        FR = bool(int(os.environ.get("FR", "0")))

        def emit_whh(th, rz, nn_, start=True, stop=False):
            for k in range(8):
                lhsT = th_lhsT(th, k)
                st = start and (k == 0)
                sp = stop and (k == 7)
                if QD:
                    for j in range(G):
                        c0 = j * 3 * GH
                        mm(rz[32 * j:32 * j + 8, :], lhsT,
                           whh[k][:, c0:c0 + 512],
                           start=st, stop=sp, tp=(0, 32 * j))
                    for j in range(G):
                        c0 = j * 3 * GH
                        mm(nn_[32 * j:32 * j + 8, 0:256], lhsT,
                           whh[k][:, c0 + 512:c0 + 768],
                           start=st, stop=sp, tp=(0, 32 * j))
                else:
                    for j in range(G):
                        c0 = j * 3 * GH
                        mm(rz[32 * j:32 * j + 8, :], lhsT,
                           whh[k][:, c0:c0 + 512],
                           start=st, stop=sp, tp=(0, 32 * j))
                        mm(nn_[32 * j:32 * j + 8, 0:256], lhsT,
                           whh[k][:, c0 + 512:c0 + 768],
                           start=st, stop=sp, tp=(0, 32 * j))

        def as_stat(p):
            return p.bitcast(f32r) if (R and p.dtype == f32) else p

        def emit_gi0(p0, rz, nn_, start=False):
            p0 = as_stat(p0)
            for j in range(G):
                c0 = j * 3 * GH
                mm(rz[32 * j:32 * j + 8, :], p0, wih0[:, c0:c0 + 512],
                   start=start, stop=False, tp=(0, 32 * j))
            for j in range(G):
                c0 = j * 3 * GH
                mm(nn_[32 * j:32 * j + 8, 256:512], p0,
                   wih0[:, c0 + 512:c0 + 768],
                   start=start, stop=False, tp=(0, 32 * j))

        def emit_gi1(p1, rz, nn_, stop=True):
            p1 = as_stat(p1)
            kp = p1.shape[0]
            for j in range(G):
                c0 = j * 3 * GH
                last = stop and (j == G - 1) and not has_bhh
                mm(rz[32 * j:32 * j + 8, :], p1, wih1[:kp, c0:c0 + 512],
                   start=False, stop=last, tp=(0, 32 * j))
            for j in range(G):
                c0 = j * 3 * GH
                last = stop and (j == G - 1) and not has_bhh
                mm(nn_[32 * j:32 * j + 8, 256:512], p1,
                   wih1[:kp, c0 + 512:c0 + 768],
                   start=False, stop=last, tp=(0, 32 * j))
            if has_bhh:
                for j in range(G):
                    mm(nn_[32 * j:32 * j + 8, 0:256], ones1[:, 0:8],
                       bhh_n[:, j * 256:(j + 1) * 256],
                       start=False, stop=(stop and j == G - 1),
                       tp=(0, 32 * j))

        def emit_gi(p0, p1, rz, nn_, start=False, stop=True):
            emit_gi0(p0, rz, nn_, start=start)
            emit_gi1(p1, rz, nn_, stop=stop)

        def gates(rz, nn_, h4_prev, warm_ps=None):
            # two pipelined column-halves (separate output tiles) so the
            # first th transpose and downstream matmuls start while half 1
            # is still on DVE/ACT. r/z/n column slices: half hf covers cols
            # hf*128:(hf+1)*128 of each 256-wide gate block.
            dummy_i = [0]

            def keep_warm(dep):
                # tiny fp32 matmul reading a just-written gates tile: keeps
                # the HAM activity window non-idle during the chain so the
                # PE clock stays at 2.4 GHz. start=False on an unwritten
                # psum region just overwrites (no bank clear).
                if warm_ps is None or not int(os.environ.get("KW", "1")):
                    return
                i = dummy_i[0]
                dummy_i[0] += 1
                mm(warm_ps[0:1, 384 + 8 * i:392 + 8 * i], dep[:, 0:1],
                   dep[:, 0:8], start=False, stop=True)

            if not G2:
                srz = gt.tile([128, 512], f32, tag="srz")
                nc.scalar.activation(srz, rz, SIG)
                t1 = gt.tile([128, 256], f32, tag="t1f")
                nc.vector.tensor_mul(t1, srz[:, 0:256], nn_[:, 0:256])
                nc.vector.tensor_add(t1, t1, nn_[:, 256:512])
                keep_warm(t1)
                sg = gt.tile([128, 256], f32, tag="sgf")
                nc.scalar.activation(sg, t1, SIG, scale=2.0)
                n_sb = gt.tile([128, 256], f32, tag="nf")
                nc.vector.tensor_scalar(n_sb, sg, 2.0, -1.0, MULT, ADD)
                h4n = []
                for hf in range(2):
                    c = hf * 128
                    d = gt.tile([128, 128], f32, tag=f"d_{hf}")
                    nc.vector.tensor_sub(d, h4_prev[hf], n_sb[:, c:c + 128])
                    nc.vector.tensor_mul(d, srz[:, 256 + c:256 + c + 128], d)
                    hn = gt.tile([128, 128], f32,
                                 tag=("h4a" if hf == 0 else "h4b"),
                                 name=f"h4nf_{hf}")
                    nc.vector.tensor_add(hn, n_sb[:, c:c + 128], d)
                    h4n.append(hn)
                keep_warm(h4n[1])
                return tuple(h4n)
            h4n = []
            for hf in range(2):
                c = hf * 128
                sr = gt.tile([128, 128], f32, tag=f"sr_{hf}")
                sz = gt.tile([128, 128], f32, tag=f"sz_{hf}")
                nc.scalar.activation(sr, rz[:, c:c + 128], SIG)
                nc.scalar.activation(sz, rz[:, 256 + c:256 + c + 128], SIG)
                t1 = gt.tile([128, 128], f32, tag=f"t1_{hf}")
                nc.vector.tensor_mul(t1, sr, nn_[:, c:c + 128])
                nc.vector.tensor_add(t1, t1, nn_[:, 256 + c:256 + c + 128])
                keep_warm(t1)
                sg = gt.tile([128, 128], f32, tag=f"sg_{hf}")
                nc.scalar.activation(sg, t1, SIG, scale=2.0)
                n_sb = gt.tile([128, 128], f32, tag=f"n_{hf}")
                nc.vector.tensor_scalar(n_sb, sg, 2.0, -1.0, MULT, ADD)
                d = gt.tile([128, 128], f32, tag=f"d_{hf}")
                nc.vector.tensor_sub(d, h4_prev[hf], n_sb)
                nc.vector.tensor_mul(d, sz, d)
                hn = gt.tile([128, 128], f32,
                             tag=("h4a" if hf == 0 else "h4b"),
                             name=f"h4n_{hf}")
                nc.vector.tensor_add(hn, n_sb, d)
                keep_warm(d)
                h4n.append(hn)
            return tuple(h4n)

        # ---------------- prologue work units ----------------
        # Emitted interleaved into the warmup loop (PI=1) so the static
        # per-engine schedule can place them in the warmup's gate-chain
        # holes; correctness is dependency-tracked either way.
        PI = bool(int(os.environ.get("PI", "0")))

        def unit_enc(m, c):
            def f():
                pr = ps_p.tile([128, 512], f32, tag="prp", name=f"pre{m}_{c}")
                for k in range(9):
                    kp = 128 if k < 8 else 1
                    mm(pr, watt[k][:kp, m * 128:(m + 1) * 128],
                       xe[c][k][:kp, :], start=(k == 0), stop=(k == 8))
                dst = projT[m].rearrange("p (b c) -> p b c", b=BL)
                nc.vector.tensor_copy(
                    dst[:, 4 * c:4 * c + 4, 0:SCE],
                    pr.rearrange("p (b c) -> p b c", b=4))
            return f

        def unit_word(m):
            def f():
                pr = ps_p.tile([128, 512], f32, tag="prp", name=f"prw{m}")
                for k in range(2):
                    kp = 128 if k == 0 else 73
                    mm(pr, wwatt[k][:kp, m * 128:(m + 1) * 128],
                       (xw0 if k == 0 else xw1)[:kp, :],
                       start=(k == 0), stop=(k == 1))
                dst = projT[m].rearrange("p (b c) -> p b c", b=BL)
                nc.vector.tensor_copy(
                    dst[:, :, SCE:SC],
                    pr.rearrange("p (b c) -> p b c", b=BL))
            return f

        def unit_p2e(b):
            def f():
                pr = ps_p.tile([128, 512], f32, tag="prp", name=f"p2e{b}")
                for k in range(8):
                    mm(pr[:, 0:OP], projT[k][:, b * SC:b * SC + SCE], wocw[k],
                       start=(k == 0), stop=False)
                mm(pr[:, 0:OP], ones1, bout_sb, start=False, stop=True)
                nc.vector.tensor_copy(p2e[:, b * OP:(b + 1) * OP], pr[:, 0:OP])
            return f

        def unit_p2w(b):
            def f():
                pr = ps_p.tile([128, 512], f32, tag="prp", name=f"p2w{b}")
                for k in range(8):
                    mm(pr[0:64, 0:OP],
                       projT[k][:, b * SC + SCE:b * SC + SC],
                       woww[k], start=(k == 0), stop=(k == 7))
                nc.vector.tensor_copy(p2w2[0:64, b * OP:(b + 1) * OP],
                                      pr[0:64, 0:OP])
            return f

        def unit_dup():
            nc.sync.dma_start(out=p2w2[64:128, :], in_=p2w2[0:64, :])

        units = []
        for m in range(8):
            units.append(unit_enc(m, 0))
            units.append(unit_enc(m, 1))
            units.append(unit_word(m))
        for b in range(BL):
            units.append(unit_p2e(b))
        for b in range(BL):
            units.append(unit_p2w(b))
        units.append(unit_dup)

        # ---------------- warmup over previous poses ----------------
        GF = bool(int(os.environ.get("GF", "0")))
        th = transpose_h(h4)
        rz = ps_g.tile([128, 512], f32, tag="rz")
        nn_ = ps_n.tile([128, 512], f32, tag="nn")
        if GF:
            # gi rounds run FIRST (carrying the psum start) so the next
            # step's gi matmuls — whose stationaries are kernel inputs —
            # can execute during the previous step's gate chain.
            wdum = ps_p.tile([1, 512], f32, tag="prp", name="wdum")
            emit_gi(posesT0[:, 0:BL], posesT1[:, 0:BL], rz, nn_,
                    start=True, stop=False)
            for t in range(PLc):
                emit_whh(th, rz, nn_, start=False, stop=True)
                rz_next = ps_g.tile([128, 512], f32, tag="rz")
                nn_next = ps_n.tile([128, 512], f32, tag="nn")
                src = t + 1 if t + 1 < PLc else PLc - 1
                emit_gi(posesT0[:, src * BL:(src + 1) * BL],
                        posesT1[:, src * BL:(src + 1) * BL],
                        rz_next, nn_next, start=True, stop=False)
                h4 = gates(rz, nn_, h4, warm_ps=wdum)
                th = transpose_h(h4)
                rz, nn_ = rz_next, nn_next
        else:
            for t in range(PLc):
                emit_whh(th, rz, nn_)
                emit_gi(posesT0[:, t * BL:(t + 1) * BL],
                        posesT1[:, t * BL:(t + 1) * BL], rz, nn_)
                if PI and t >= PLc // 2:
                    # start only once the big input DMAs had time to land,
                    # else the unit matmuls stall the PE FIFO head and
                    # block the next warmup step behind them.
                    left = PLc - t
                    n_take = (len(units) + left - 1) // left
                    for _ in range(min(n_take, len(units))):
                        units.pop(0)()
                rz_next = ps_g.tile([128, 512], f32, tag="rz")
                nn_next = ps_n.tile([128, 512], f32, tag="nn")
                h4 = gates(rz, nn_, h4, warm_ps=rz_next)
                th = transpose_h(h4)
                rz, nn_ = rz_next, nn_next

        # drain any prologue units not interleaved into the warmup
        for u in units:
            u()
        units = []

        # ---------------- main loop ----------------
        # first step's gru accumulation from the last previous pose
        if GF:
            # its gi rounds were already emitted in the warmup tail
            emit_whh(th, rz, nn_, start=False, stop=True)
        else:
            emit_whh(th, rz, nn_)
            emit_gi(posesT0[:, (PLc - 1) * BL:PLc * BL],
                    posesT1[:, (PLc - 1) * BL:PLc * BL], rz, nn_)
        h4 = gates(rz, nn_, h4)
        th = transpose_h(h4)

        for t in range(T):
            # 1. middle: scores + pose-woh (contract th(t))
            sc = ps_s.tile([128, 384], f32, tag="sc")
            pp = ps_p.tile([128, 512], f32, tag="prp", name=f"pp{t}")
            for k in range(8):
                lhsT = th_lhsT(th, k)
                pv = projT[k].rearrange("p (b c) -> p b c", b=BL)
                wv = woh2[k].rearrange("p (r c) -> p r c", r=2)
                for j in range(G):
                    mm(sc[32 * j:32 * j + 8, :].rearrange(
                        "p (r c) -> p r c", r=2),
                       lhsT, pv[:, j:j + 5:4, :],
                       start=(k == 0), stop=(k == 7), tp=(0, 32 * j))
                    mm(pp[32 * j:32 * j + 8, 0:2 * OP].rearrange(
                        "p (r c) -> p r c", r=2),
                       lhsT, wv,
                       start=(k == 0), stop=False, tp=(0, 32 * j))

            # 2. next step's W_hh rounds run while softmax happens
            if t < T - 1:
                rz = ps_g.tile([128, 512], f32, tag="rz")
                nn_ = ps_n.tile([128, 512], f32, tag="nn")
                emit_whh(th, rz, nn_)

            # 3. softmax (no max-subtraction; scores bounded ~21); sums on
            # DVE so the ACT queue is just the 4 exps.
            w_es = [gt.tile([128, 128], f32, tag=f"w_e{rr}",
                            name=f"w_e{rr}_{t}") for rr in range(2)]
            w_ws = [gt.tile([128, 64], f32, tag=f"w_w{rr}",
                            name=f"w_w{rr}_{t}") for rr in range(2)]
            sums = [gt.tile([128, 1], f32, tag=f"sum{i}",
                            name=f"sum{i}_{t}") for i in range(4)]
            rinv = [gt.tile([128, 1], f32, tag=f"rinv{i}",
                            name=f"rinv{i}_{t}") for i in range(4)]
            AX = mybir.AxisListType.X
            if SM2:
                for rr in range(2):
                    nc.scalar.activation(w_es[rr],
                                         sc[:, rr * SC:rr * SC + SCE], EXP)
                    nc.scalar.activation(w_ws[rr],
                                         sc[:, rr * SC + SCE:(rr + 1) * SC],
                                         EXP)
                for rr in range(2):
                    nc.vector.reduce_sum(out=sums[rr], in_=w_es[rr], axis=AX)
                    nc.vector.reciprocal(rinv[rr], sums[rr])
                    nc.vector.reduce_sum(out=sums[2 + rr], in_=w_ws[rr],
                                         axis=AX)
                    nc.vector.reciprocal(rinv[2 + rr], sums[2 + rr])
            else:
                for rr in range(2):
                    nc.scalar.activation(w_es[rr],
                                         sc[:, rr * SC:rr * SC + SCE], EXP,
                                         accum_out=sums[rr])
                    nc.scalar.activation(w_ws[rr],
                                         sc[:, rr * SC + SCE:(rr + 1) * SC],
                                         EXP, accum_out=sums[2 + rr])
                for i in range(4):
                    nc.vector.reciprocal(rinv[i], sums[i])

            # 4. normalize+cast, transpose -> stationaries; enc folds start
            # as soon as both wte are ready, word folds carry the stop.
            wte = []
            w_wb = gt.tile([128, 128], bf16, tag="w_wb")
            for rr in range(2):
                w_eb = gt.tile([128, 128], bf16, tag=f"w_eb{rr}")
                nc.vector.tensor_scalar_mul(w_eb, w_es[rr], rinv[rr])
                tpb = ps_t.tile([128, 128], bf16, tag="tp", name=f"te{rr}_{t}")
                nc.tensor.transpose(tpb, w_eb, identb)
                sb = gt.tile([128, 128], bf16, tag=f"wte{rr}")
                nc.vector.tensor_copy(sb, tpb)
                wte.append(sb)
            for rr in range(2):
                nc.vector.tensor_scalar_mul(w_wb[:, rr * 64:(rr + 1) * 64],
                                            w_ws[rr], rinv[2 + rr])

            if FR:
                # 5a. enc folds
                for j in range(G):
                    for rr in range(2):
                        b = 4 * rr + j
                        mm(pp[32 * j:32 * j + 8, rr * OP:rr * OP + OP],
                           wte[rr][:, 32 * j:32 * j + 8],
                           p2e[:, b * OP:(b + 1) * OP],
                           start=False, stop=False, tp=(0, 32 * j))

            tpw = ps_t.tile([128, 128], bf16, tag="tp", name=f"tw_{t}")
            nc.tensor.transpose(tpw, w_wb, identb)
            wtw = gt.tile([128, 128], bf16, tag="wtw")
            nc.vector.tensor_copy(wtw, tpw)

            if FR:
                # 5b. word folds (carry the accumulation-group stop)
                for j in range(G):
                    for rr in range(2):
                        b = 4 * rr + j
                        mm(pp[32 * j:32 * j + 8, rr * OP:rr * OP + OP],
                           wtw[rr * 64:(rr + 1) * 64, 32 * j:32 * j + 8],
                           p2w2[rr * 64:(rr + 1) * 64, b * OP:(b + 1) * OP],
                           start=False, stop=(j == G - 1 and rr == 1),
                           tp=(rr * 64, 32 * j))
            else:
                for j in range(G):
                    for rr in range(2):
                        b = 4 * rr + j
                        mm(pp[32 * j:32 * j + 8, rr * OP:rr * OP + OP],
                           wte[rr][:, 32 * j:32 * j + 8],
                           p2e[:, b * OP:(b + 1) * OP],
                           start=False, stop=False, tp=(0, 32 * j))
                        mm(pp[32 * j:32 * j + 8, rr * OP:rr * OP + OP],
                           wtw[rr * 64:(rr + 1) * 64, 32 * j:32 * j + 8],
                           p2w2[rr * 64:(rr + 1) * 64, b * OP:(b + 1) * OP],
                           start=False, stop=(j == G - 1 and rr == 1),
                           tp=(rr * 64, 32 * j))

            # 6. pose copies (bf16 first: it gates the transposes)
            pose_b = gt.tile([128, 2 * OP], bf16, tag="pose_b")
            nc.vector.tensor_copy(pose_b, pp[:, 0:2 * OP])
            pose_f = gt.tile([128, 2 * OP], f32, tag="pose_f")
            nc.vector.tensor_copy(pose_f, pp[:, 0:2 * OP])

            # 7. output DMA (2 per step; rows 33j+4rr are valid batches)
            for rr in range(2):
                nc.sync.dma_start(
                    out=outs["poses"][t, 4 * rr:4 * rr + 4, :],
                    in_=pose_f[4 * rr:4 * rr + 100:33,
                               rr * OP:rr * OP + O])

            if t == T - 1:
                break

            # 8. pose transposes; gi kk=0 starts after the big transposes.
            # PE transposes do NOT count as PE-busy for the HAM clock
            # governor, so thread real (tiny) matmuls through this region
            # to keep the clock at 2.4 GHz.
            pt0 = gt.tile([128, 8], f32 if R else bf16, tag="pt0")
            pt1 = gt.tile([8, 8], f32 if R else bf16, tag="pt1")
            if int(os.environ.get("PW", "0")):
                mm(pp[0:1, 448:456], pose_b[:, 0:1], pose_b[:, 0:8],
                   start=False, stop=True)
            if FR:
                for rr in range(2):
                    tpb = ps_t.tile([128, 128], bf16, tag="tp",
                                    name=f"tp{rr}_{t}")
                    nc.tensor.transpose(tpb, pose_b[:, rr * OP:rr * OP + 128],
                                        identb)
                    nc.vector.tensor_copy(pt0[:, 4 * rr:4 * rr + 4],
                                          tpb[:, 4 * rr:4 * rr + 100:33])
                emit_gi0(pt0, rz, nn_)
                for rr in range(2):
                    tpt = ps_t.tile([128, 128], bf16, tag="tp",
                                    name=f"tpt{rr}_{t}")
                    nc.tensor.transpose(
                        tpt[0:8, :], pose_b[:, rr * OP + 128:rr * OP + 136],
                        identb)
                    nc.vector.tensor_copy(pt1[0:8, 4 * rr:4 * rr + 4],
                                          tpt[0:8, 4 * rr:4 * rr + 100:33])
                emit_gi1(pt1, rz, nn_)
            else:
                for rr in range(2):
                    tpt = ps_t.tile([128, 128], bf16, tag="tp",
                                    name=f"tpt{rr}_{t}")
                    nc.tensor.transpose(
                        tpt[0:8, :], pose_b[:, rr * OP + 128:rr * OP + 136],
                        identb)
                    nc.vector.tensor_copy(pt1[0:8, 4 * rr:4 * rr + 4],
                                          tpt[0:8, 4 * rr:4 * rr + 100:33])
                    tpb = ps_t.tile([128, 128], bf16, tag="tp",
                                    name=f"tp{rr}_{t}")
                    nc.tensor.transpose(tpb, pose_b[:, rr * OP:rr * OP + 128],
                                        identb)
                    nc.vector.tensor_copy(pt0[:, 4 * rr:4 * rr + 4],
                                          tpb[:, 4 * rr:4 * rr + 100:33])
                    if int(os.environ.get("PW", "0")):
                        mm(pp[0:1, 456 + 16 * rr:464 + 16 * rr],
                           pt0[:, 0:1], pt0[:, 0:8], start=False, stop=True)
                emit_gi(pt0, pt1, rz, nn_)

            # 9. next step's gate nonlinearity + th transposes
            h4 = gates(rz, nn_, h4, warm_ps=pp)
            th = transpose_h(h4)


def _build(T, PLc, has_bhh):
    import concourse.tile as tile
    from concourse import bacc, mybir

    f32 = mybir.dt.float32
    bf16 = mybir.dt.bfloat16
    nc = bacc.Bacc("TRN2", target_bir_lowering=False, debug=False,
                   num_devices=NCORES)
    ins = {}

    def di(name, shape, dt=bf16):
        ins[name] = nc.dram_tensor(name, list(shape), dt,
                                   kind="ExternalInput").ap()

    di("xt_enc", (E + 1, BL * S))
    di("xt_word", (201, BL * WL))
    di("eht", (E + 1, BL))
    di("poses_t", (136, PLc * BL))
    di("whh_t", (H, 3 * H))
    di("wih_t", (136, 3 * H))
    di("woh2_t", (H, 2 * OP))
    di("woc_t", (H, OP))
    di("wow_t", (H, OP))
    di("bout", (1, OP))
    di("watt_t", (E + 1, H))
    di("wwatt_t", (201, H))
    di("wed_t", (E + 1, H))
    if has_bhh:
        di("bhh_n", (1, H))
    outs = {"poses": nc.dram_tensor("poses", [T, BL, O], f32,
                                    kind="ExternalOutput").ap()}
    with tile.TileContext(nc) as tc:
        _body(tc, outs, ins, T, PLc, has_bhh)
    nc.compile()
    return nc


def _host_prep(inputs, PLc, has_bhh):
    """Per-core input maps (host transposes + weight prep), bf16."""
    import ml_dtypes
    bf = ml_dtypes.bfloat16

    enc = np.asarray(inputs["encoder_states"], np.float32)
    ehid = np.asarray(inputs["encoder_hidden"], np.float32)
    pp = np.asarray(inputs["previous_poses"], np.float32)
    words = np.asarray(inputs["words"], np.float32)
    W_ed, b_ed = np.asarray(inputs["W_ed"], np.float32), np.asarray(inputs["b_ed"], np.float32)
    W_att, b_att = np.asarray(inputs["W_att"], np.float32), np.asarray(inputs["b_att"], np.float32)
    W_watt, b_watt = np.asarray(inputs["W_watt"], np.float32), np.asarray(inputs["b_watt"], np.float32)
    W_ih, W_hh = np.asarray(inputs["W_ih"], np.float32), np.asarray(inputs["W_hh"], np.float32)
    b_ih, b_hh = np.asarray(inputs["b_ih"], np.float32), np.asarray(inputs["b_hh"], np.float32)
    W_out, b_out = np.asarray(inputs["W_out"], np.float32), np.asarray(inputs["b_out"], np.float32)

    gc = _group_cols()
    bihg, bhhg = b_ih[gc], b_hh[gc]
    # ones-row bias: rz slice gets b_ih+b_hh, n slice gets b_ih only
    gb = bihg + bhhg
    bhh_n = np.zeros(H, np.float32)
    for j in range(G):
        c0 = j * 3 * GH
        gb[c0 + 512:c0 + 768] = bihg[c0 + 512:c0 + 768]
        bhh_n[j * GH:(j + 1) * GH] = bhhg[c0 + 512:c0 + 768]

    whh_t = W_hh.T[:, gc]
    wih_t = np.zeros((136, 3 * H), np.float32)
    wih_t[:O] = W_ih.T[:, gc]
    wih_t[O] = gb

    woh_t = np.zeros((H, OP), np.float32)
    woh_t[:, :O] = W_out[:, :H].T
    woh2_t = np.concatenate([woh_t, woh_t], 1)
    woc_t = np.zeros((H, OP), np.float32)
    woc_t[:, :O] = W_out[:, H:2 * H].T
    wow_t = np.zeros((H, OP), np.float32)
    wow_t[:, :O] = W_out[:, 2 * H:].T
    bout = np.zeros((1, OP), np.float32)
    bout[0, :O] = b_out
    bout[0, O] = 1.0  # ones-col: pose col 135 = sum(softmax) = 1

    watt_t = np.concatenate([W_att.T, b_att[None, :]], 0)
    wwatt_t = np.concatenate([W_watt.T, b_watt[None, :]], 0)
    wed_t = np.concatenate([W_ed.T, b_ed[None, :]], 0)

    shared = dict(whh_t=whh_t, wih_t=wih_t,
                  woh2_t=woh2_t, woc_t=woc_t, wow_t=wow_t, bout=bout,
                  watt_t=watt_t, wwatt_t=wwatt_t, wed_t=wed_t)
    if has_bhh:
        shared["bhh_n"] = bhh_n[None, :]
    shared = {k: np.ascontiguousarray(v.astype(bf)) for k, v in shared.items()}

    in_maps = []
    for c in range(NCORES):
        bs = slice(c * BL, (c + 1) * BL)
        xt_enc = np.zeros((E + 1, BL * S), np.float32)
        xt_enc[:E] = np.transpose(enc[:, bs, :], (2, 1, 0)).reshape(E, BL * S)
        xt_enc[E] = 1.0
        xt_word = np.zeros((201, BL * WL), np.float32)
        xt_word[:200] = np.transpose(words[:, bs, :], (2, 1, 0)).reshape(200, BL * WL)
        xt_word[200] = 1.0
        eh = np.transpose(ehid[:, bs, :], (1, 0, 2)).reshape(BL, E)
        eht = np.zeros((E + 1, BL), np.float32)
        eht[:E] = eh.T
        eht[E] = 1.0
        poses_t = np.zeros((136, PLc, BL), np.float32)
        poses_t[:O] = np.transpose(pp[:, bs, :], (2, 0, 1))
        poses_t[O] = 1.0
        poses_t = poses_t.reshape(136, PLc * BL)
        m = dict(xt_enc=xt_enc, xt_word=xt_word, eht=eht, poses_t=poses_t)
        m = {k: np.ascontiguousarray(v.astype(bf)) for k, v in m.items()}
        m.update(shared)
        in_maps.append(m)
    return in_maps


def kernel(**inputs):
    from concourse.bass_utils import run_bass_kernel_spmd

    T = int(inputs["real_poses_len"])
    PLc = int(inputs["previous_poses"].shape[0])
    has_bhh = bool(np.any(np.asarray(inputs["b_hh"], np.float32) != 0))
    key = (T, PLc, has_bhh)
    if key not in _progs:
        _progs[key] = _build(T, PLc, has_bhh)
    nc = _progs[key]
    in_maps = _host_prep(inputs, PLc, has_bhh)
    trace = bool(int(os.environ.get("KERNEL_TRACE", "0")))
    res = run_bass_kernel_spmd(nc, in_maps, core_ids=list(range(NCORES)),
                               trace=trace)
    if trace:
        kernel.last_exec_time_ns = res.exec_time_ns
        kernel.last_mean_exec_time_ns = res.mean_exec_time_ns
    out = np.concatenate([res.results[c]["poses"] for c in range(NCORES)], axis=1)
    return out.astype(np.float32)


# revision 44
# speedup vs baseline: 1.0151x; 1.0151x over previous
"""Trainium2 Bass kernel for nn_Decoder (GRU decoder with dual attention).

Strategy (8 NeuronCores, batch-parallel, zero collectives). v2 rewrite of
the baseline, targeting the measured bottleneck: per-matmul LDWEIGHTS /
dispatch cost (~80ns) x 242 matmuls/step, plus a ~4us per-step PE idle
gap that kept HAM at half clock.

Key changes vs baseline:
  - M=8 stationaries everywhere (LDWEIGHTS cost ~ columns): th slices
    [128,8] feed col-group-tiled matmuls at tile_position (0,32j).
  - rr-pairs merged into single matmuls via 2-block (3D) APs: scores
    N=384, pose-woh N=272 (woh stored duplicated), output DMAs 2/step.
  - All gate biases folded into the W_ih ones-row (row 135); optional
    b_hh n-part mini-round only compiled when b_hh != 0.
  - Softmax without max-subtraction (|scores| <= ~21 for this problem,
    exp fits fp32 with huge margin), per-segment independent tiles so
    the 4 exps pipeline instead of serializing, normalization fused into
    the bf16 cast (tensor_scalar_mul with per-partition 1/sum).
  - tanh(x) == 2*sigmoid(2x)-1 so ACT uses only Sigmoid+Exp tables.
  - Cross-step overlap: next step's W_hh rounds are emitted right after
    this step's score matmuls so the PE stays busy during softmax; PSUM
    rz/nn pools are double-buffered. Eliminates the HAM re-throttle gap.
  - p2e column 135 := 1.0 (b_out ones-col) so the transposed pose
    carries the gi-bias ones row for free (softmax weights sum to 1).

Layouts (per core, BL=8 batches):
  h4  [128, 256] fp32: row 32*j+b = h[b, j*256:(j+1)*256]  (b<8 valid)
  th[half] [128,128] bf16: th[half][r, 32*q+b] = h[b, q*256+half*128+r]
    => contraction tile k uses lhsT th[k%2][:, 32*(k//2) : 32*(k//2)+8]
  gate cols grouped: group j = [r_j | z_j | n_j] each 256 wide;
    rz psum [128,512] = r|z for group j at rows 32j; nn psum [128,512]:
    cols 0:256 = gh_n, 256:512 = gi_n.
  projT[k] [128, 8*192] bf16: projT[k][r, b*192+c]: c<128 enc proj at
    s=c; c in 128:192 word proj at wl=c-128; h-dim k*128+r; bias folded.
  p2e [128, 8*136]: p2e[s, b*136+o] = (W_oc @ enc_proj + b_out)[o],
    col o=135 is 1.0 (ones trick).  p2w2 [128, 8*136]: word analogue
    duplicated on partitions 0:64 and 64:128.
"""

import os
import sys

sys.path.insert(0, "/opt/trn_rl_repo")

import numpy as np

S, B, E, H, O, WL, PL = 128, 64, 1024, 1024, 135, 64, 32
NCORES = 8
BL = B // NCORES          # 8 batches per core
G = 4                     # gate column groups
GH = H // G               # 256 hidden dims per group
OP = 136                  # padded pose dim (135 + ones col)
SCE, SCW = S, WL
SC = SCE + SCW            # 192 score cols per batch

_progs = {}


def _group_cols():
    """Column permutation of the 3H gate dim into G groups of [r|z|n]."""
    cols = []
    for j in range(G):
        h0 = j * GH
        cols.extend(range(h0, h0 + GH))
        cols.extend(range(H + h0, H + h0 + GH))
        cols.extend(range(2 * H + h0, 2 * H + h0 + GH))
    return np.asarray(cols)


def _body(tc, outs, ins, T, PLc, has_bhh):
    from concourse import mybir
    from concourse.masks import make_identity

    nc = tc.nc
    f32 = mybir.dt.float32
    bf16 = mybir.dt.bfloat16
    EXP = mybir.ActivationFunctionType.Exp
    SIG = mybir.ActivationFunctionType.Sigmoid
    MULT = mybir.AluOpType.mult
    ADD = mybir.AluOpType.add

    def mm(out, lhsT, rhs, start, stop, tp=None):
        nc.tensor.matmul(out, lhsT, rhs, start=start, stop=stop,
                         tile_position=tp, skip_group_check=True)

    import contextlib
    ctx = contextlib.ExitStack()
    with ctx:
        wp = ctx.enter_context(tc.tile_pool(name="wp", bufs=1))
        work = ctx.enter_context(tc.tile_pool(name="work", bufs=2))
        gt = ctx.enter_context(
            tc.tile_pool(name="gt", bufs=int(os.environ.get("GB", "2"))))
        ps_g = ctx.enter_context(tc.tile_pool(name="ps_g", bufs=2, space="PSUM"))
        ps_n = ctx.enter_context(tc.tile_pool(name="ps_n", bufs=2, space="PSUM"))
        ps_s = ctx.enter_context(tc.tile_pool(name="ps_s", bufs=1, space="PSUM"))
        ps_p = ctx.enter_context(tc.tile_pool(name="ps_p", bufs=1, space="PSUM"))
        ps_t = ctx.enter_context(tc.tile_pool(name="ps_t", bufs=2, space="PSUM"))

        # ---------------- persistent weights ----------------
        whh = []
        for k in range(8):
            t = wp.tile([128, 3 * H], bf16, tag=f"whh{k}")
            nc.sync.dma_start(out=t, in_=ins["whh_t"][k * 128:(k + 1) * 128, :])
            whh.append(t)
        wih0 = wp.tile([128, 3 * H], bf16, tag="wih0")
        nc.sync.dma_start(out=wih0, in_=ins["wih_t"][0:128, :])
        wih1 = wp.tile([8, 3 * H], bf16, tag="wih1")
        nc.sync.dma_start(out=wih1, in_=ins["wih_t"][128:136, :])
        woh2 = []
        for k in range(8):
            t = wp.tile([128, 2 * OP], bf16, tag=f"woh2_{k}")
            nc.sync.dma_start(out=t, in_=ins["woh2_t"][k * 128:(k + 1) * 128, :])
            woh2.append(t)
        posesT0 = wp.tile([128, PLc * BL], bf16, tag="posesT0")
        nc.sync.dma_start(out=posesT0, in_=ins["poses_t"][0:128, :])
        posesT1 = wp.tile([8, PLc * BL], bf16, tag="posesT1")
        nc.sync.dma_start(out=posesT1, in_=ins["poses_t"][128:136, :])
        if has_bhh:
            bhh_n = wp.tile([1, H], bf16, tag="bhh_n")
            nc.sync.dma_start(out=bhh_n, in_=ins["bhh_n"][:, :])
        bout_sb = wp.tile([1, OP], bf16, tag="bout_sb")
        nc.sync.dma_start(out=bout_sb, in_=ins["bout"][:, :])

        ident = wp.tile([128, 128], f32, tag="ident")
        make_identity(nc, ident[:, :])
        identb = wp.tile([128, 128], bf16, tag="identb")
        nc.vector.tensor_copy(identb, ident)
        ones1 = wp.tile([1, 128], bf16, tag="ones1")
        nc.vector.memset(ones1, 1.0)

        projT = [wp.tile([128, BL * SC], bf16, tag=f"projT{m}", name=f"projT{m}")
                 for m in range(8)]
        p2e = wp.tile([128, BL * OP], bf16, tag="p2e")
        p2w2 = wp.tile([128, BL * OP], bf16, tag="p2w2")

        # ---------------- prologue input DMAs (start early) --------------
        # enc activations: [1025, 1024] in 2 col-chunks of 512, 9 row tiles
        xe = [[None] * 9 for _ in range(2)]
        for c in range(2):
            for k in range(9):
                kp = 128 if k < 8 else 1
                t = work.tile([128, 512], bf16, tag="xe", bufs=18,
                              name=f"xe{c}_{k}")
                nc.sync.dma_start(
                    out=t[:kp, :],
                    in_=ins["xt_enc"][k * 128:k * 128 + kp,
                                      c * 512:(c + 1) * 512])
                xe[c][k] = t
        xw0 = work.tile([128, 512], bf16, tag="xw0", bufs=1)
        nc.sync.dma_start(out=xw0, in_=ins["xt_word"][0:128, :])
        xw1 = work.tile([73, 512], bf16, tag="xw1", bufs=1)
        nc.sync.dma_start(out=xw1, in_=ins["xt_word"][128:201, :])
        # watt resident during prologue: 9 x [128, 1024]
        watt = []
        for k in range(9):
            kp = 128 if k < 8 else 1
            t = work.tile([128, H], bf16, tag="watt", bufs=9, name=f"watt{k}")
            nc.sync.dma_start(out=t[:kp, :],
                              in_=ins["watt_t"][k * 128:k * 128 + kp, :])
            watt.append(t)
        wwatt = []
        for k in range(2):
            kp = 128 if k == 0 else 73
            t = work.tile([128, H], bf16, tag="wwatt", bufs=2, name=f"wwatt{k}")
            nc.sync.dma_start(out=t[:kp, :],
                              in_=ins["wwatt_t"][k * 128:k * 128 + kp, :])
            wwatt.append(t)
        wocw = [work.tile([128, OP], bf16, tag="wocw", bufs=16, name=f"wocw{k}")
                for k in range(8)]
        for k in range(8):
            nc.sync.dma_start(out=wocw[k], in_=ins["woc_t"][k * 128:(k + 1) * 128, :])
        woww = [work.tile([128, OP], bf16, tag="wocw", bufs=16, name=f"woww{k}")
                for k in range(8)]
        for k in range(8):
            nc.sync.dma_start(out=woww[k], in_=ins["wow_t"][k * 128:(k + 1) * 128, :])
        ehk = []
        for k in range(9):
            kp = 128 if k < 8 else 1
            t = work.tile([128, BL], bf16, tag="ehk", bufs=9, name=f"ehk{k}")
            nc.sync.dma_start(out=t[:kp, :],
                              in_=ins["eht"][k * 128:k * 128 + kp, :])
            ehk.append(t)
        wed = []
        for k in range(9):
            kp = 128 if k < 8 else 1
            t = work.tile([128, H], bf16, tag="wed", bufs=9, name=f"wed{k}")
            nc.sync.dma_start(out=t[:kp, :],
                              in_=ins["wed_t"][k * 128:k * 128 + kp, :])
            wed.append(t)

        # ---------------- h0 ----------------
        h0p = ps_s.tile([128, 384], f32, tag="sc", name="h0p")
        for k in range(9):
            kp = 128 if k < 8 else 1
            for j in range(G):
                mm(h0p[32 * j:32 * j + 8, 0:256], ehk[k][:kp, :],
                   wed[k][:kp, j * GH:(j + 1) * GH],
                   start=(k == 0), stop=(k == 8), tp=(0, 32 * j))
        h4 = (gt.tile([128, 128], f32, tag="h4a", name="h4a_init"),
              gt.tile([128, 128], f32, tag="h4b", name="h4b_init"))
        nc.vector.tensor_copy(h4[0], h0p[:, 0:128])
        nc.vector.tensor_copy(h4[1], h0p[:, 128:256])

        # ---------------- recurrent helpers ----------------
        f32r = mybir.dt.float32r
        R = bool(int(os.environ.get("R32", "0")))

        def th_lhsT(th, k):
            q = k // 2
            sl = th[k % 2][:, 32 * q:32 * q + 8]
            return sl.bitcast(f32r) if R else sl

        def transpose_h(h4t):
            th = []
            for half in range(2):
                tp = ps_t.tile([128, 128], f32, tag="tp")
                nc.tensor.transpose(tp, h4t[half], ident)
                sb = gt.tile([128, 128], f32 if R else bf16, tag=f"th{half}")
                nc.vector.tensor_copy(sb, tp)
                th.append(sb)
            return th

        QD = bool(int(os.environ.get("QD", "1")))
        G2 = bool(int(os.environ.get("G2", "1")))
        SM2 = bool(int(os.environ.get("SM2", "1")))
        FR = bool(int(os.environ.get("FR", "0")))

        def emit_whh(th, rz, nn_, start=True, stop=False):
            for k in range(8):
                lhsT = th_lhsT(th, k)
                st = start and (k == 0)
                sp = stop and (k == 7)
                if QD:
                    for j in range(G):
                        c0 = j * 3 * GH
                        mm(rz[32 * j:32 * j + 8, :], lhsT,
                           whh[k][:, c0:c0 + 512],
                           start=st, stop=sp, tp=(0, 32 * j))
                    for j in range(G):
                        c0 = j * 3 * GH
                        mm(nn_[32 * j:32 * j + 8, 0:256], lhsT,
                           whh[k][:, c0 + 512:c0 + 768],
                           start=st, stop=sp, tp=(0, 32 * j))
                else:
                    for j in range(G):
                        c0 = j * 3 * GH
                        mm(rz[32 * j:32 * j + 8, :], lhsT,
                           whh[k][:, c0:c0 + 512],
                           start=st, stop=sp, tp=(0, 32 * j))
                        mm(nn_[32 * j:32 * j + 8, 0:256], lhsT,
                           whh[k][:, c0 + 512:c0 + 768],
                           start=st, stop=sp, tp=(0, 32 * j))

        def as_stat(p):
            return p.bitcast(f32r) if (R and p.dtype == f32) else p

        def emit_gi0(p0, rz, nn_, start=False):
            p0 = as_stat(p0)
            for j in range(G):
                c0 = j * 3 * GH
                mm(rz[32 * j:32 * j + 8, :], p0, wih0[:, c0:c0 + 512],
                   start=start, stop=False, tp=(0, 32 * j))
            for j in range(G):
                c0 = j * 3 * GH
                mm(nn_[32 * j:32 * j + 8, 256:512], p0,
                   wih0[:, c0 + 512:c0 + 768],
                   start=start, stop=False, tp=(0, 32 * j))

        def emit_gi1(p1, rz, nn_, stop=True):
            p1 = as_stat(p1)
            kp = p1.shape[0]
            for j in range(G):
                c0 = j * 3 * GH
                last = stop and (j == G - 1) and not has_bhh
                mm(rz[32 * j:32 * j + 8, :], p1, wih1[:kp, c0:c0 + 512],
                   start=False, stop=last, tp=(0, 32 * j))
            for j in range(G):
                c0 = j * 3 * GH
                last = stop and (j == G - 1) and not has_bhh
                mm(nn_[32 * j:32 * j + 8, 256:512], p1,
                   wih1[:kp, c0 + 512:c0 + 768],
                   start=False, stop=last, tp=(0, 32 * j))
            if has_bhh:
                for j in range(G):
                    mm(nn_[32 * j:32 * j + 8, 0:256], ones1[:, 0:8],
                       bhh_n[:, j * 256:(j + 1) * 256],
                       start=False, stop=(stop and j == G - 1),
                       tp=(0, 32 * j))

        def emit_gi(p0, p1, rz, nn_, start=False, stop=True):
            emit_gi0(p0, rz, nn_, start=start)
            emit_gi1(p1, rz, nn_, stop=stop)

        def gates(rz, nn_, h4_prev, warm_ps=None):
            # two pipelined column-halves (separate output tiles) so the
            # first th transpose and downstream matmuls start while half 1
            # is still on DVE/ACT. r/z/n column slices: half hf covers cols
            # hf*128:(hf+1)*128 of each 256-wide gate block.
            dummy_i = [0]

            def keep_warm(dep):
                # tiny fp32 matmul reading a just-written gates tile: keeps
                # the HAM activity window non-idle during the chain so the
                # PE clock stays at 2.4 GHz. start=False on an unwritten
                # psum region just overwrites (no bank clear).
                if warm_ps is None or not int(os.environ.get("KW", "1")):
                    return
                i = dummy_i[0]
                dummy_i[0] += 1
                mm(warm_ps[0:1, 384 + 8 * i:392 + 8 * i], dep[:, 0:1],
                   dep[:, 0:8], start=False, stop=True)

            if not G2:
                srz = gt.tile([128, 512], f32, tag="srz")
                nc.scalar.activation(srz, rz, SIG)
                t1 = gt.tile([128, 256], f32, tag="t1f")
                nc.vector.tensor_mul(t1, srz[:, 0:256], nn_[:, 0:256])
                nc.vector.tensor_add(t1, t1, nn_[:, 256:512])
                keep_warm(t1)
                sg = gt.tile([128, 256], f32, tag="sgf")
                nc.scalar.activation(sg, t1, SIG, scale=2.0)
                n_sb = gt.tile([128, 256], f32, tag="nf")
                nc.vector.tensor_scalar(n_sb, sg, 2.0, -1.0, MULT, ADD)
                h4n = []
                for hf in range(2):
                    c = hf * 128
                    d = gt.tile([128, 128], f32, tag=f"d_{hf}")
                    nc.vector.tensor_sub(d, h4_prev[hf], n_sb[:, c:c + 128])
                    nc.vector.tensor_mul(d, srz[:, 256 + c:256 + c + 128], d)
                    hn = gt.tile([128, 128], f32,
                                 tag=("h4a" if hf == 0 else "h4b"),
                                 name=f"h4nf_{hf}")
                    nc.vector.tensor_add(hn, n_sb[:, c:c + 128], d)
                    h4n.append(hn)
                keep_warm(h4n[1])
                return tuple(h4n)
            h4n = []
            for hf in range(2):
                c = hf * 128
                sr = gt.tile([128, 128], f32, tag=f"sr_{hf}")
                sz = gt.tile([128, 128], f32, tag=f"sz_{hf}")
                nc.scalar.activation(sr, rz[:, c:c + 128], SIG)
                nc.scalar.activation(sz, rz[:, 256 + c:256 + c + 128], SIG)
                t1 = gt.tile([128, 128], f32, tag=f"t1_{hf}")
                nc.vector.tensor_mul(t1, sr, nn_[:, c:c + 128])
                nc.vector.tensor_add(t1, t1, nn_[:, 256 + c:256 + c + 128])
                keep_warm(t1)
                sg = gt.tile([128, 128], f32, tag=f"sg_{hf}")
                nc.scalar.activation(sg, t1, SIG, scale=2.0)
                n_sb = gt.tile([128, 128], f32, tag=f"n_{hf}")
                nc.vector.tensor_scalar(n_sb, sg, 2.0, -1.0, MULT, ADD)
                d = gt.tile([128, 128], f32, tag=f"d_{hf}")
                nc.vector.tensor_sub(d, h4_prev[hf], n_sb)
                nc.vector.tensor_mul(d, sz, d)
                hn = gt.tile([128, 128], f32,
                             tag=("h4a" if hf == 0 else "h4b"),
                             name=f"h4n_{hf}")
                nc.vector.tensor_add(hn, n_sb, d)
                keep_warm(d)
                h4n.append(hn)
            return tuple(h4n)

        # ---------------- prologue work units ----------------
        # Emitted interleaved into the warmup loop (PI=1) so the static
        # per-engine schedule can place them in the warmup's gate-chain
        # holes; correctness is dependency-tracked either way.
        PI = bool(int(os.environ.get("PI", "0")))

        def unit_enc(m, c):
            def f():
                pr = ps_p.tile([128, 512], f32, tag="prp", name=f"pre{m}_{c}")
                for k in range(9):
                    kp = 128 if k < 8 else 1
                    mm(pr, watt[k][:kp, m * 128:(m + 1) * 128],
                       xe[c][k][:kp, :], start=(k == 0), stop=(k == 8))
                dst = projT[m].rearrange("p (b c) -> p b c", b=BL)
                nc.vector.tensor_copy(
                    dst[:, 4 * c:4 * c + 4, 0:SCE],
                    pr.rearrange("p (b c) -> p b c", b=4))
            return f

        def unit_word(m):
            def f():
                pr = ps_p.tile([128, 512], f32, tag="prp", name=f"prw{m}")
                for k in range(2):
                    kp = 128 if k == 0 else 73
                    mm(pr, wwatt[k][:kp, m * 128:(m + 1) * 128],
                       (xw0 if k == 0 else xw1)[:kp, :],
                       start=(k == 0), stop=(k == 1))
                dst = projT[m].rearrange("p (b c) -> p b c", b=BL)
                nc.vector.tensor_copy(
                    dst[:, :, SCE:SC],
                    pr.rearrange("p (b c) -> p b c", b=BL))
            return f

        def unit_p2e(b):
            def f():
                pr = ps_p.tile([128, 512], f32, tag="prp", name=f"p2e{b}")
                for k in range(8):
                    mm(pr[:, 0:OP], projT[k][:, b * SC:b * SC + SCE], wocw[k],
                       start=(k == 0), stop=False)
                mm(pr[:, 0:OP], ones1, bout_sb, start=False, stop=True)
                nc.vector.tensor_copy(p2e[:, b * OP:(b + 1) * OP], pr[:, 0:OP])
            return f

        def unit_p2w(b):
            def f():
                pr = ps_p.tile([128, 512], f32, tag="prp", name=f"p2w{b}")
                for k in range(8):
                    mm(pr[0:64, 0:OP],
                       projT[k][:, b * SC + SCE:b * SC + SC],
                       woww[k], start=(k == 0), stop=(k == 7))
                nc.vector.tensor_copy(p2w2[0:64, b * OP:(b + 1) * OP],
                                      pr[0:64, 0:OP])
            return f

        def unit_dup():
            nc.sync.dma_start(out=p2w2[64:128, :], in_=p2w2[0:64, :])

        units = []
        for m in range(8):
            units.append(unit_enc(m, 0))
            units.append(unit_enc(m, 1))
            units.append(unit_word(m))
        for b in range(BL):
            units.append(unit_p2e(b))
        for b in range(BL):
            units.append(unit_p2w(b))
        units.append(unit_dup)

        # ---------------- warmup over previous poses ----------------
        GF = bool(int(os.environ.get("GF", "0")))
        th = transpose_h(h4)
        rz = ps_g.tile([128, 512], f32, tag="rz")
        nn_ = ps_n.tile([128, 512], f32, tag="nn")
        if GF:
            # gi rounds run FIRST (carrying the psum start) so the next
            # step's gi matmuls — whose stationaries are kernel inputs —
            # can execute during the previous step's gate chain.
            wdum = ps_p.tile([1, 512], f32, tag="prp", name="wdum")
            emit_gi(posesT0[:, 0:BL], posesT1[:, 0:BL], rz, nn_,
                    start=True, stop=False)
            for t in range(PLc):
                emit_whh(th, rz, nn_, start=False, stop=True)
                rz_next = ps_g.tile([128, 512], f32, tag="rz")
                nn_next = ps_n.tile([128, 512], f32, tag="nn")
                src = t + 1 if t + 1 < PLc else PLc - 1
                emit_gi(posesT0[:, src * BL:(src + 1) * BL],
                        posesT1[:, src * BL:(src + 1) * BL],
                        rz_next, nn_next, start=True, stop=False)
                h4 = gates(rz, nn_, h4, warm_ps=wdum)
                th = transpose_h(h4)
                rz, nn_ = rz_next, nn_next
        else:
            for t in range(PLc):
                emit_whh(th, rz, nn_)
                emit_gi(posesT0[:, t * BL:(t + 1) * BL],
                        posesT1[:, t * BL:(t + 1) * BL], rz, nn_)
                if PI and t >= PLc // 2:
                    # start only once the big input DMAs had time to land,
                    # else the unit matmuls stall the PE FIFO head and
                    # block the next warmup step behind them.
                    left = PLc - t
                    n_take = (len(units) + left - 1) // left
                    for _ in range(min(n_take, len(units))):
                        units.pop(0)()
                rz_next = ps_g.tile([128, 512], f32, tag="rz")
                nn_next = ps_n.tile([128, 512], f32, tag="nn")
                h4 = gates(rz, nn_, h4, warm_ps=rz_next)
                th = transpose_h(h4)
                rz, nn_ = rz_next, nn_next

        # drain any prologue units not interleaved into the warmup
        for u in units:
            u()
        units = []

        # ---------------- main loop ----------------
        # first step's gru accumulation from the last previous pose
        if GF:
            # its gi rounds were already emitted in the warmup tail
            emit_whh(th, rz, nn_, start=False, stop=True)
        else:
            emit_whh(th, rz, nn_)
            emit_gi(posesT0[:, (PLc - 1) * BL:PLc * BL],
                    posesT1[:, (PLc - 1) * BL:PLc * BL], rz, nn_)
        h4 = gates(rz, nn_, h4)
        th = transpose_h(h4)

        for t in range(T):
            # 1. middle: scores + pose-woh (contract th(t))
            sc = ps_s.tile([128, 384], f32, tag="sc")
            pp = ps_p.tile([128, 512], f32, tag="prp", name=f"pp{t}")
            for k in range(8):
                lhsT = th_lhsT(th, k)
                pv = projT[k].rearrange("p (b c) -> p b c", b=BL)
                wv = woh2[k].rearrange("p (r c) -> p r c", r=2)
                for j in range(G):
                    mm(sc[32 * j:32 * j + 8, :].rearrange(
                        "p (r c) -> p r c", r=2),
                       lhsT, pv[:, j:j + 5:4, :],
                       start=(k == 0), stop=(k == 7), tp=(0, 32 * j))
                    mm(pp[32 * j:32 * j + 8, 0:2 * OP].rearrange(
                        "p (r c) -> p r c", r=2),
                       lhsT, wv,
                       start=(k == 0), stop=False, tp=(0, 32 * j))

            # 2. next step's W_hh rounds run while softmax happens
            if t < T - 1:
                rz = ps_g.tile([128, 512], f32, tag="rz")
                nn_ = ps_n.tile([128, 512], f32, tag="nn")
                emit_whh(th, rz, nn_)

            # 3. softmax (no max-subtraction; scores bounded ~21); sums on
            # DVE so the ACT queue is just the 4 exps.
            w_es = [gt.tile([128, 128], f32, tag=f"w_e{rr}",
                            name=f"w_e{rr}_{t}") for rr in range(2)]
            w_ws = [gt.tile([128, 64], f32, tag=f"w_w{rr}",
                            name=f"w_w{rr}_{t}") for rr in range(2)]
            sums = [gt.tile([128, 1], f32, tag=f"sum{i}",
                            name=f"sum{i}_{t}") for i in range(4)]
            rinv = [gt.tile([128, 1], f32, tag=f"rinv{i}",
                            name=f"rinv{i}_{t}") for i in range(4)]
            AX = mybir.AxisListType.X
            if SM2:
                for rr in range(2):
                    nc.scalar.activation(w_es[rr],
                                         sc[:, rr * SC:rr * SC + SCE], EXP)
                    nc.scalar.activation(w_ws[rr],
                                         sc[:, rr * SC + SCE:(rr + 1) * SC],
                                         EXP)
                for rr in range(2):
                    nc.vector.reduce_sum(out=sums[rr], in_=w_es[rr], axis=AX)
                    nc.vector.reciprocal(rinv[rr], sums[rr])
                    nc.vector.reduce_sum(out=sums[2 + rr], in_=w_ws[rr],
                                         axis=AX)
                    nc.vector.reciprocal(rinv[2 + rr], sums[2 + rr])
            else:
                for rr in range(2):
                    nc.scalar.activation(w_es[rr],
                                         sc[:, rr * SC:rr * SC + SCE], EXP,
                                         accum_out=sums[rr])
                    nc.scalar.activation(w_ws[rr],
                                         sc[:, rr * SC + SCE:(rr + 1) * SC],
                                         EXP, accum_out=sums[2 + rr])
                for i in range(4):
                    nc.vector.reciprocal(rinv[i], sums[i])

            # 4. normalize+cast, transpose -> stationaries; enc folds start
            # as soon as both wte are ready, word folds carry the stop.
            wte = []
            w_wb = gt.tile([128, 128], bf16, tag="w_wb")
            for rr in range(2):
                w_eb = gt.tile([128, 128], bf16, tag=f"w_eb{rr}")
                nc.vector.tensor_scalar_mul(w_eb, w_es[rr], rinv[rr])
                tpb = ps_t.tile([128, 128], bf16, tag="tp", name=f"te{rr}_{t}")
                nc.tensor.transpose(tpb, w_eb, identb)
                sb = gt.tile([128, 128], bf16, tag=f"wte{rr}")
                nc.vector.tensor_copy(sb, tpb)
                wte.append(sb)
            for rr in range(2):
                nc.vector.tensor_scalar_mul(w_wb[:, rr * 64:(rr + 1) * 64],
                                            w_ws[rr], rinv[2 + rr])

            if FR:
                # 5a. enc folds
                for j in range(G):
                    for rr in range(2):
                        b = 4 * rr + j
                        mm(pp[32 * j:32 * j + 8, rr * OP:rr * OP + OP],
                           wte[rr][:, 32 * j:32 * j + 8],
                           p2e[:, b * OP:(b + 1) * OP],
                           start=False, stop=False, tp=(0, 32 * j))

            tpw = ps_t.tile([128, 128], bf16, tag="tp", name=f"tw_{t}")
            nc.tensor.transpose(tpw, w_wb, identb)
            wtw = gt.tile([128, 128], bf16, tag="wtw")
            nc.vector.tensor_copy(wtw, tpw)

            if FR:
                # 5b. word folds (carry the accumulation-group stop)
                for j in range(G):
                    for rr in range(2):
                        b = 4 * rr + j
                        mm(pp[32 * j:32 * j + 8, rr * OP:rr * OP + OP],
                           wtw[rr * 64:(rr + 1) * 64, 32 * j:32 * j + 8],
                           p2w2[rr * 64:(rr + 1) * 64, b * OP:(b + 1) * OP],
                           start=False, stop=(j == G - 1 and rr == 1),
                           tp=(rr * 64, 32 * j))
            else:
                for j in range(G):
                    for rr in range(2):
                        b = 4 * rr + j
                        mm(pp[32 * j:32 * j + 8, rr * OP:rr * OP + OP],
                           wte[rr][:, 32 * j:32 * j + 8],
                           p2e[:, b * OP:(b + 1) * OP],
                           start=False, stop=False, tp=(0, 32 * j))
                        mm(pp[32 * j:32 * j + 8, rr * OP:rr * OP + OP],
                           wtw[rr * 64:(rr + 1) * 64, 32 * j:32 * j + 8],
                           p2w2[rr * 64:(rr + 1) * 64, b * OP:(b + 1) * OP],
                           start=False, stop=(j == G - 1 and rr == 1),
                           tp=(rr * 64, 32 * j))

            # 6. pose copies (bf16 first: it gates the transposes)
            pose_b = gt.tile([128, 2 * OP], bf16, tag="pose_b")
            nc.vector.tensor_copy(pose_b, pp[:, 0:2 * OP])
            pose_f = gt.tile([128, 2 * OP], f32, tag="pose_f")
            nc.vector.tensor_copy(pose_f, pp[:, 0:2 * OP])

            # 7. output DMA (2 per step; rows 33j+4rr are valid batches)
            for rr in range(2):
                nc.sync.dma_start(
                    out=outs["poses"][t, 4 * rr:4 * rr + 4, :],
                    in_=pose_f[4 * rr:4 * rr + 100:33,
                               rr * OP:rr * OP + O])

            if t == T - 1:
                break

            # 8. pose transposes; gi kk=0 starts after the big transposes.
            # PE transposes do NOT count as PE-busy for the HAM clock
            # governor, so thread real (tiny) matmuls through this region
            # to keep the clock at 2.4 GHz.
            pt0 = gt.tile([128, 8], f32 if R else bf16, tag="pt0")
            pt1 = gt.tile([8, 8], f32 if R else bf16, tag="pt1")
            if int(os.environ.get("PW", "0")):
                mm(pp[0:1, 448:456], pose_b[:, 0:1], pose_b[:, 0:8],
                   start=False, stop=True)
            if FR:
                for rr in range(2):
                    tpb = ps_t.tile([128, 128], bf16, tag="tp",
                                    name=f"tp{rr}_{t}")
                    nc.tensor.transpose(tpb, pose_b[:, rr * OP:rr * OP + 128],
                                        identb)
                    nc.vector.tensor_copy(pt0[:, 4 * rr:4 * rr + 4],
                                          tpb[:, 4 * rr:4 * rr + 100:33])
                emit_gi0(pt0, rz, nn_)
                for rr in range(2):
                    tpt = ps_t.tile([128, 128], bf16, tag="tp",
                                    name=f"tpt{rr}_{t}")
                    nc.tensor.transpose(
                        tpt[0:8, :], pose_b[:, rr * OP + 128:rr * OP + 136],
                        identb)
                    nc.vector.tensor_copy(pt1[0:8, 4 * rr:4 * rr + 4],
                                          tpt[0:8, 4 * rr:4 * rr + 100:33])
                emit_gi1(pt1, rz, nn_)
            else:
                for rr in range(2):
                    tpt = ps_t.tile([128, 128], bf16, tag="tp",
                                    name=f"tpt{rr}_{t}")
                    nc.tensor.transpose(
                        tpt[0:8, :], pose_b[:, rr * OP + 128:rr * OP + 136],
                        identb)
                    nc.vector.tensor_copy(pt1[0:8, 4 * rr:4 * rr + 4],
                                          tpt[0:8, 4 * rr:4 * rr + 100:33])
                    tpb = ps_t.tile([128, 128], bf16, tag="tp",
                                    name=f"tp{rr}_{t}")
                    nc.tensor.transpose(tpb, pose_b[:, rr * OP:rr * OP + 128],
                                        identb)
                    nc.vector.tensor_copy(pt0[:, 4 * rr:4 * rr + 4],
                                          tpb[:, 4 * rr:4 * rr + 100:33])
                    if int(os.environ.get("PW", "0")):
                        mm(pp[0:1, 456 + 16 * rr:464 + 16 * rr],
                           pt0[:, 0:1], pt0[:, 0:8], start=False, stop=True)
                emit_gi(pt0, pt1, rz, nn_)

            # 9. next step's gate nonlinearity + th transposes
            h4 = gates(rz, nn_, h4, warm_ps=pp)
            th = transpose_h(h4)


def _build(T, PLc, has_bhh):
    import concourse.tile as tile
    from concourse import bacc, mybir

    f32 = mybir.dt.float32
    bf16 = mybir.dt.bfloat16
    nc = bacc.Bacc("TRN2", target_bir_lowering=False, debug=False,
                   num_devices=NCORES)
    ins = {}

    def di(name, shape, dt=bf16):
        ins[name] = nc.dram_tensor(name, list(shape), dt,
                                   kind="ExternalInput").ap()

    di("xt_enc", (E + 1, BL * S))
    di("xt_word", (201, BL * WL))
    di("eht", (E + 1, BL))
    di("poses_t", (136, PLc * BL))
    di("whh_t", (H, 3 * H))
    di("wih_t", (136, 3 * H))
    di("woh2_t", (H, 2 * OP))
    di("woc_t", (H, OP))
    di("wow_t", (H, OP))
    di("bout", (1, OP))
    di("watt_t", (E + 1, H))
    di("wwatt_t", (201, H))
    di("wed_t", (E + 1, H))
    if has_bhh:
        di("bhh_n", (1, H))
    outs = {"poses": nc.dram_tensor("poses", [T, BL, O], f32,
                                    kind="ExternalOutput").ap()}
    with tile.TileContext(nc) as tc:
        _body(tc, outs, ins, T, PLc, has_bhh)
    nc.compile()
    return nc


def _host_prep(inputs, PLc, has_bhh):
    """Per-core input maps (host transposes + weight prep), bf16."""
    import ml_dtypes
    bf = ml_dtypes.bfloat16

    enc = np.asarray(inputs["encoder_states"], np.float32)
    ehid = np.asarray(inputs["encoder_hidden"], np.float32)
    pp = np.asarray(inputs["previous_poses"], np.float32)
    words = np.asarray(inputs["words"], np.float32)
    W_ed, b_ed = np.asarray(inputs["W_ed"], np.float32), np.asarray(inputs["b_ed"], np.float32)
    W_att, b_att = np.asarray(inputs["W_att"], np.float32), np.asarray(inputs["b_att"], np.float32)
    W_watt, b_watt = np.asarray(inputs["W_watt"], np.float32), np.asarray(inputs["b_watt"], np.float32)
    W_ih, W_hh = np.asarray(inputs["W_ih"], np.float32), np.asarray(inputs["W_hh"], np.float32)
    b_ih, b_hh = np.asarray(inputs["b_ih"], np.float32), np.asarray(inputs["b_hh"], np.float32)
    W_out, b_out = np.asarray(inputs["W_out"], np.float32), np.asarray(inputs["b_out"], np.float32)

    gc = _group_cols()
    bihg, bhhg = b_ih[gc], b_hh[gc]
    # ones-row bias: rz slice gets b_ih+b_hh, n slice gets b_ih only
    gb = bihg + bhhg
    bhh_n = np.zeros(H, np.float32)
    for j in range(G):
        c0 = j * 3 * GH
        gb[c0 + 512:c0 + 768] = bihg[c0 + 512:c0 + 768]
        bhh_n[j * GH:(j + 1) * GH] = bhhg[c0 + 512:c0 + 768]

    whh_t = W_hh.T[:, gc]
    wih_t = np.zeros((136, 3 * H), np.float32)
    wih_t[:O] = W_ih.T[:, gc]
    wih_t[O] = gb

    woh_t = np.zeros((H, OP), np.float32)
    woh_t[:, :O] = W_out[:, :H].T
    woh2_t = np.concatenate([woh_t, woh_t], 1)
    woc_t = np.zeros((H, OP), np.float32)
    woc_t[:, :O] = W_out[:, H:2 * H].T
    wow_t = np.zeros((H, OP), np.float32)
    wow_t[:, :O] = W_out[:, 2 * H:].T
    bout = np.zeros((1, OP), np.float32)
    bout[0, :O] = b_out
    bout[0, O] = 1.0  # ones-col: pose col 135 = sum(softmax) = 1

    watt_t = np.concatenate([W_att.T, b_att[None, :]], 0)
    wwatt_t = np.concatenate([W_watt.T, b_watt[None, :]], 0)
    wed_t = np.concatenate([W_ed.T, b_ed[None, :]], 0)

    shared = dict(whh_t=whh_t, wih_t=wih_t,
                  woh2_t=woh2_t, woc_t=woc_t, wow_t=wow_t, bout=bout,
                  watt_t=watt_t, wwatt_t=wwatt_t, wed_t=wed_t)
    if has_bhh:
        shared["bhh_n"] = bhh_n[None, :]
    shared = {k: np.ascontiguousarray(v.astype(bf)) for k, v in shared.items()}

    in_maps = []
    for c in range(NCORES):
        bs = slice(c * BL, (c + 1) * BL)
        xt_enc = np.zeros((E + 1, BL * S), np.float32)
        xt_enc[:E] = np.transpose(enc[:, bs, :], (2, 1, 0)).reshape(E, BL * S)
        xt_enc[E] = 1.0
        xt_word = np.zeros((201, BL * WL), np.float32)
        xt_word[:200] = np.transpose(words[:, bs, :], (2, 1, 0)).reshape(200, BL * WL)
        xt_word[200] = 1.0
        eh = np.transpose(ehid[:, bs, :], (1, 0, 2)).reshape(BL, E)
        eht = np.zeros((E + 1, BL), np.float32)
        eht[:E] = eh.T
        eht[E] = 1.0
        poses_t = np.zeros((136, PLc, BL), np.float32)
        poses_t[:O] = np.transpose(pp[:, bs, :], (2, 0, 1))
        poses_t[O] = 1.0
        poses_t = poses_t.reshape(136, PLc * BL)
        m = dict(xt_enc=xt_enc, xt_word=xt_word, eht=eht, poses_t=poses_t)
        m = {k: np.ascontiguousarray(v.astype(bf)) for k, v in m.items()}
        m.update(shared)
        in_maps.append(m)
    return in_maps


def kernel(**inputs):
    from concourse.bass_utils import run_bass_kernel_spmd

    T = int(inputs["real_poses_len"])
    PLc = int(inputs["previous_poses"].shape[0])
    has_bhh = bool(np.any(np.asarray(inputs["b_hh"], np.float32) != 0))
    key = (T, PLc, has_bhh)
    if key not in _progs:
        _progs[key] = _build(T, PLc, has_bhh)
    nc = _progs[key]
    in_maps = _host_prep(inputs, PLc, has_bhh)
    trace = bool(int(os.environ.get("KERNEL_TRACE", "0")))
    res = run_bass_kernel_spmd(nc, in_maps, core_ids=list(range(NCORES)),
                               trace=trace)
    if trace:
        kernel.last_exec_time_ns = res.exec_time_ns
        kernel.last_mean_exec_time_ns = res.mean_exec_time_ns
    out = np.concatenate([res.results[c]["poses"] for c in range(NCORES)], axis=1)
    return out.astype(np.float32)
